# revision 32
# baseline (speedup 1.0000x reference)
"""Cross-attention kernel for Trainium2, 8 NeuronCores.

Problem: b=4, s=2048, d_model=1024, n_heads=16 (head_dim=64), fp32.
  out = softmax((q@Wq) (k@Wk)^T / sqrt(64) + mask) @ (v@Wv) @ Wo + bo

Sharding: core c handles batch c//2 and head-group c%2 (8 heads, 512
projection columns). Each core computes a partial output (s, 1024) =
(its heads' attention output) @ Wo[rows of its heads]; the host sums
the two partials per batch and adds bo.

v2 design (single fused pipeline, ACT-engine bound):
  * All matmul operands bf16 (inputs converted host-side); PSUM fp32.
  * S^T tiles for a HEAD PAIR computed concurrently via PE row tiling
    (contraction=64 each: head A rows 0-63, head B rows 64-127), into
    one [128, 1024] PSUM pair tile -> one exp() per j-tile.
  * AV keeps the ones-column trick (M=65: numerator^T rows + softmax
    denominator row); V stored head-interleaved with mask-scaled ones
    columns written by DVE (no N=8 tail matmuls).
  * K/V/Q projections pipelined INTO the attention stream (j-group
    granularity for ic=0; Q(ic+1) and O-proj(ic) groups interleaved as
    PE filler), AttnOut staged in SBUF (no DRAM bounce), so the scalar
    engine (exp: 33.5M elem/core ~ 220us floor) is saturated end2end.
  * Softmax division: ot PSUM copied to SBUF immediately (frees the
    single-buffered ot banks), reciprocal of the denominator row is
    partition-broadcast via a DRAM bounce, multiply on DVE.
PSUM: st pair tiles 2x[128,1024] (4 banks) + ot A/B 2x[128,512]
(2 banks, single-buffered) + proj/O-proj ring 2x[128,512] (2 banks).
"""

import collections

import numpy as np

import concourse.bass as bass
import concourse.tile as tile
from concourse import mybir
from concourse.bass_utils import run_bass_kernel_spmd

P = 128
S = 2048          # sequence length
DIN = 1024        # model dim
C = 512           # projection columns per core (8 heads * 64)
NHC = 8           # heads per core
HD = 64           # head dim
VW = NHC * (HD + 1)   # 520: head-interleaved V width incl. ones columns
NIC = S // 512    # 4 i-chunks
NJT = S // P      # 16 j-tiles
LAG = 2           # S^T -> AV software-pipeline distance (j-tiles)
F32 = mybir.dt.float32
BF16 = mybir.dt.bfloat16


def _build_kernel():
    nc = bass.Bass("TRN2", target_bir_lowering=False, debug=False)

    qT = nc.dram_tensor("qT", [DIN, S], BF16, kind="ExternalInput").ap()
    kT = nc.dram_tensor("kT", [DIN, S], BF16, kind="ExternalInput").ap()
    vT = nc.dram_tensor("vT", [DIN, S], BF16, kind="ExternalInput").ap()
    wq = nc.dram_tensor("wq", [DIN, C], BF16, kind="ExternalInput").ap()
    wk = nc.dram_tensor("wk", [DIN, C], BF16, kind="ExternalInput").ap()
    wv = nc.dram_tensor("wv", [DIN, C], BF16, kind="ExternalInput").ap()
    wo = nc.dram_tensor("wo", [C, DIN], BF16, kind="ExternalInput").ap()
    bq = nc.dram_tensor("bq", [C], F32, kind="ExternalInput").ap()
    bk = nc.dram_tensor("bk", [C], F32, kind="ExternalInput").ap()
    bv = nc.dram_tensor("bv", [C], F32, kind="ExternalInput").ap()
    mm = nc.dram_tensor("mm", [S], F32, kind="ExternalInput").ap()
    onec = nc.dram_tensor("onec", [NHC], F32, kind="ExternalInput").ap()
    y = nc.dram_tensor("y", [S, DIN], F32, kind="ExternalOutput").ap()

    rcp_dram = nc.dram_tensor("rcp_st", [32, 512], F32).ap()  # 1/denom bounce

    with tile.TileContext(nc) as tc:
        _body(tc, y, rcp_dram, qT, kT, vT, wq, wk, wv, wo, bq, bk, bv, mm,
              onec)
    return nc


def _bcast_rows(ap, parts):
    """AP reading a 1-D (or row) DRAM region broadcast over `parts`
    partitions."""
    return bass.AP(tensor=ap.tensor, offset=ap.offset,
                   ap=[[0, parts]] + list(ap.ap))


def _blocks(ap2d, nblk, blk_stride, width, blk_off=0):
    """[p, nblk, width] AP over a 2-D [p, F] slice: blocks of `width`
    elements every `blk_stride`, starting at `blk_off`."""
    return bass.AP(tensor=ap2d.tensor, offset=ap2d.offset + blk_off,
                   ap=[list(ap2d.ap[0]), [blk_stride, nblk], [1, width]])


def _strided(ap2d, start, stride, count):
    """[p, count] AP: one element every `stride`, starting at `start`."""
    return bass.AP(tensor=ap2d.tensor, offset=ap2d.offset + start,
                   ap=[list(ap2d.ap[0]), [stride, count]])


def _xsrc(x_dram, col0, cols):
    """[128, 8, cols] AP over x_dram [1024, S]: partition = row%128,
    block = row//128, innermost = cols starting at col0."""
    return bass.AP(tensor=x_dram.tensor, offset=x_dram.offset + col0,
                   ap=[[S, P], [P * S, 8], [1, cols]])


def _body(tc, y, rcp_dram, qT, kT, vT, wq, wk, wv, wo, bq, bk, bv, mm, onec):
    nc = tc.nc

    with (
        tc.tile_pool(name="wpool", bufs=1) as wpool,
        tc.tile_pool(name="big", bufs=1) as big,
        tc.tile_pool(name="xin", bufs=2) as xin,
        tc.tile_pool(name="expool", bufs=8) as expool,
        tc.tile_pool(name="cppool", bufs=2) as cppool,
        tc.tile_pool(name="bcpool", bufs=2) as bcpool,
        tc.tile_pool(name="rcpp", bufs=2) as rcpp,
        tc.tile_pool(name="ytp", bufs=2) as ytp,
        tc.tile_pool(name="stp", bufs=2, space="PSUM") as stp,
        tc.tile_pool(name="otp", bufs=1, space="PSUM") as otp,
        tc.tile_pool(name="pjp", bufs=2, space="PSUM") as pjp,
    ):
        # ---------------- static tiles + weight DMAs ----------------
        # DMA transfers serialize in emission order (single-queue FIFO in
        # the model; shared HBM BW on hw), so emission order = arrival
        # priority: K-projection operands (wk+xk) gate the very first
        # matmuls, then Q, then V; wo can land tens of microseconds in.
        wk_sb = wpool.tile([P, 8, C], BF16)
        nc.sync.dma_start(out=wk_sb, in_=wk.rearrange("(t p) c -> p t c", p=P))

        # split per i-chunk / j-group / head-pair: tile-granular RAW
        # tracking would otherwise serialize readers of one chunk behind
        # writers of another
        qhT = [big.tile([P, 4, 512], BF16, name=f"qhT{i}") for i in range(4)]
        khT = [big.tile([P, 4, 512], BF16, name=f"khT{i}") for i in range(4)]
        vh = [big.tile([P, 4, VW], BF16, name=f"vh{i}") for i in range(4)]
        stg = [big.tile([P, S], BF16, name=f"stg{i}") for i in range(4)]

        # ---------------- helper closures ----------------
        def dma_x(tag, x_dram, col0, cols=512):
            xt = xin.tile([P, 8, cols], BF16, tag=tag, name=f"x{tag}")
            nc.sync.dma_start(out=xt, in_=_xsrc(x_dram, col0, cols))
            return xt

        def qk_group(dst, w_sb, b_sb, xt, io):
            """dst[:, io, :] = (W[:, io-block]^T @ x) + bias"""
            ps = pjp.tile([P, 512], F32, name="pspj")
            for kt in range(8):
                nc.tensor.matmul(
                    ps,
                    (w_sb[:, kt, io * P:(io + 1) * P]),
                    (xt[:, kt, :]),
                    start=(kt == 0),
                    stop=(kt == 7),
                )
            nc.vector.tensor_scalar_add(
                out=dst[:, io, :],
                in0=ps,
                scalar1=b_sb[:, io:io + 1],
            )

        def v_group(jt, xt):
            """vh[:, jt, :] = interleave((x_jt @ Wv) + bv, ones) * mask"""
            ji = jt % 4
            ps = pjp.tile([P, 512], F32, name="pspj")
            for kt in range(8):
                nc.tensor.matmul(
                    ps,
                    (xt[:, kt, ji * P:(ji + 1) * P]),
                    (wv_sb[:, kt, :]),
                    start=(kt == 0),
                    stop=(kt == 7),
                )
            v2d = vh[jt // 4][:, jt % 4, :]
            numer = _blocks(v2d, NHC, HD + 1, HD)
            nc.vector.tensor_tensor(
                out=numer,
                in0=_blocks(ps, NHC, HD, HD),
                in1=_blocks(bvb, NHC, HD, HD),
                op=mybir.AluOpType.add,
            )
            nc.vector.tensor_scalar_mul(
                out=numer,
                in0=numer,
                scalar1=mm_sb[:, jt:jt + 1],
            )
            nc.vector.tensor_scalar_mul(
                out=_strided(v2d, HD, HD + 1, NHC),
                in0=ones8,
                scalar1=mm_sb[:, jt:jt + 1],
            )

        def st_step(ic, p, jt):
            """S^T pair tile for heads (2p, 2p+1) at (ic, jt) + exp."""
            st = stp.tile([P, 1024], F32, name="st")
            for u in range(2):
                nc.tensor.matmul(
                    st[:, u * 512:(u + 1) * 512],
                    (khT[jt // 4][u * HD:(u + 1) * HD, p,
                                  (jt % 4) * P:(jt % 4 + 1) * P]),
                    (qhT[ic][u * HD:(u + 1) * HD, p, :]),
                    start=True,
                    stop=True,
                )
            ex = expool.tile([P, 1024], BF16, name="ex")
            nc.scalar.activation(
                out=ex,
                in_=st,
                func=mybir.ActivationFunctionType.Exp,
                scale=float(HD) ** -0.5,
            )
            return ex

        def av_step(ex, otA, otB, p, jt):
            for u, ot in ((0, otA), (1, otB)):
                h = 2 * p + u
                nc.tensor.matmul(
                    ot[0:HD + 1, :],
                    (vh[jt // 4][:, jt % 4, h * (HD + 1):(h + 1) * (HD + 1)]),
                    (ex[:, u * 512:(u + 1) * 512]),
                    start=(jt == 0),
                    stop=(jt == NJT - 1),
                )

        def division(ic, p, otA, otB, last=False):
            """stg[:, p, ic-block] = numerators / denominator.

            Normally the full-tile copies run first so the single-buffered
            ot banks free ASAP (they gate the next pair's AV matmuls); on
            the last pair nothing follows, so the reciprocals read PSUM
            directly to start the DMA broadcast bounce sooner."""
            r2 = rcpp.tile([1, 1024], F32, name="r2")
            cA = cppool.tile([HD + 1, 512], F32, tag="cA", name="cA")
            cB = cppool.tile([HD + 1, 512], F32, tag="cB", name="cB")
            if last:
                nc.vector.reciprocal(out=r2[:, 0:512], in_=otA[HD:HD + 1, :])
                nc.vector.reciprocal(out=r2[:, 512:1024], in_=otB[HD:HD + 1, :])
                nc.vector.tensor_copy(out=cA, in_=otA[0:HD + 1, :])
                nc.vector.tensor_copy(out=cB, in_=otB[0:HD + 1, :])
            else:
                nc.vector.tensor_copy(out=cA, in_=otA[0:HD + 1, :])
                nc.vector.tensor_copy(out=cB, in_=otB[0:HD + 1, :])
                nc.vector.reciprocal(out=r2[:, 0:512], in_=cA[HD:HD + 1, :])
                nc.vector.reciprocal(out=r2[:, 512:1024], in_=cB[HD:HD + 1, :])
            slot = 2 * (ic * 4 + p)
            rows = rcp_dram[slot:slot + 2, :]
            # SWDGE (gpsimd-issued) DMAs mid-stream: the sem-wait on the
            # reciprocals would otherwise block the SP DMA-issue FIFO,
            # delaying every bulk transfer queued behind it. On the last
            # pair SP is empty, so its lower-latency HWDGE path wins.
            dma = nc.sync.dma_start if last else nc.gpsimd.dma_start
            dma(out=rows, in_=r2)
            bc = bcpool.tile([HD, 1024], F32, name="bc")
            # one DMA: each denominator row broadcast over 64 partitions
            # (A's in cols 0-511, B's in 512-1023, both at base partition 0)
            dma(out=bc,
                in_=bass.AP(tensor=rows.tensor, offset=rows.offset,
                            ap=[[0, HD], [512, 2], [1, 512]]))
            icb = slice(ic * 512, (ic + 1) * 512)
            # one multiply on DVE, one on the otherwise-idle Pool engine:
            # they run in parallel, halving the division tail.
            nc.vector.tensor_tensor(
                out=stg[p][0:HD, icb], in0=cA[0:HD, :], in1=bc[:, 0:512],
                op=mybir.AluOpType.mult)
            nc.gpsimd.tensor_tensor(
                out=stg[p][HD:P, icb], in0=cB[0:HD, :], in1=bc[:, 512:1024],
                op=mybir.AluOpType.mult)

        def o_mm(ps, it, ec, ft0, ft1):
            for ft in range(ft0, ft1):
                nc.tensor.matmul(
                    ps,
                    (stg[ft][:, it * P:(it + 1) * P]),
                    (wo_sb[:, ft, ec * 512:(ec + 1) * 512]),
                    start=(ft == 0),
                    stop=(ft == 3),
                )

        def o_drain(ps, it, ec, act=False):
            """act=True drains via the scalar engine (idle in the epilogue;
            Copy is in every activation table set so no table reload)."""
            yt = ytp.tile([P, 512], F32, name="yt")
            if act:
                nc.scalar.activation(
                    out=yt, in_=ps, func=mybir.ActivationFunctionType.Copy)
            else:
                nc.vector.tensor_copy(out=yt, in_=ps)
            nc.sync.dma_start(
                out=y[it * P:(it + 1) * P, ec * 512:(ec + 1) * 512], in_=yt)

        def o_psum(n):
            if n % 2 == 0:
                # reuse the st tag's (idle in the epilogue) ring slots
                return stp.tile([P, 512], F32, name="pso", tag="st")
            return pjp.tile([P, 512], F32, name="pspj")

        def o_psum_ep(n):
            # epilogue: also borrow the freed ot banks -> 4 independent slots
            kind = ("st", "pj", "A", "B")[n % 4]
            if kind == "st":
                return stp.tile([P, 512], F32, name="pso", tag="st")
            if kind == "pj":
                return pjp.tile([P, 512], F32, name="pspj")
            return otp.tile([P, 512], F32, name=f"pso{kind}", tag=kind)

        def o_group(it, ec, act=False):
            """y[it-block, ec-block] = AttnOut[it] @ Wo[:, ec-block]."""
            ps = o_psum(1 if not act else 0)
            o_mm(ps, it, ec, 0, 4)
            o_drain(ps, it, ec, act)

        # ---------------- prologue ----------------
        xk = dma_x("xk", kT, 0)
        bk_sb = wpool.tile([P, 4], F32)
        nc.sync.dma_start(out=bk_sb, in_=bk.rearrange("(t p) -> p t", p=P))
        wq_sb = wpool.tile([P, 8, C], BF16)
        nc.sync.dma_start(out=wq_sb, in_=wq.rearrange("(t p) c -> p t c", p=P))
        xq = dma_x("xq", qT, 0)
        bq_sb = wpool.tile([P, 4], F32)
        nc.sync.dma_start(out=bq_sb, in_=bq.rearrange("(t p) -> p t", p=P))
        wv_sb = wpool.tile([P, 8, C], BF16)
        nc.sync.dma_start(out=wv_sb, in_=wv.rearrange("(t p) c -> p t c", p=P))
        xv = dma_x("xv", vT, 0)
        bvb = wpool.tile([P, C], F32)
        nc.sync.dma_start(out=bvb, in_=_bcast_rows(bv, P))
        mm_sb = wpool.tile([P, NJT], F32)
        nc.sync.dma_start(out=mm_sb, in_=mm.rearrange("(t p) -> p t", p=P))
        ones8 = wpool.tile([P, NHC], F32)
        nc.sync.dma_start(out=ones8, in_=_bcast_rows(onec, P))
        wo_sb = wpool.tile([P, 4, DIN], BF16)
        nc.sync.dma_start(out=wo_sb, in_=wo.rearrange("(t p) c -> p t c", p=P))
        for io in range(4):
            qk_group(khT[0], wk_sb, bk_sb, xk, io)
        for io in range(4):
            qk_group(qhT[0], wq_sb, bq_sb, xq, io)
        for jt in range(4):
            v_group(jt, xv)

        # ---------------- fused attention pipeline ----------------
        fillers = collections.deque()
        # ic0/pair0 K+V projection stream, consumed 2 per step (8 groups
        # per j-group of 4 steps -> each j-group ready exactly in time).
        kvx = [None, None, None, None]

        def mk_kv_dma(jg):
            def f():
                kvx[jg] = (dma_x("xk", kT, jg * 512), dma_x("xv", vT, jg * 512))
            return f

        def mk_k(jg, io):
            return lambda: qk_group(khT[jg], wk_sb, bk_sb, kvx[jg][0], io)

        def mk_v(jg, jt):
            return lambda: v_group(jt, kvx[jg][1])

        ic0_fill = collections.deque()
        for jg in range(1, 4):
            ic0_fill.append(mk_kv_dma(jg))
            for io in range(4):
                ic0_fill.append(mk_k(jg, io))
            for jt in range(jg * 4, jg * 4 + 4):
                ic0_fill.append(mk_v(jg, jt))

        qx = [None]

        def mk_q_dma(ic):
            def f():
                qx[0] = dma_x("xq", qT, ic * 512)
            return f

        def mk_q(ic, io):
            return lambda: qk_group(qhT[ic], wq_sb, bq_sb, qx[0], io)

        def mk_o(it, ec, act=False):
            return lambda: o_group(it, ec, act)

        pending = collections.deque()  # (ex, otA, otB, p, jt, ic)
        step = [0]

        def flush_one():
            ex, otA, otB, p, jt, pic = pending.popleft()
            av_step(ex, otA, otB, p, jt)
            if jt == NJT - 1:
                last = pic == NIC - 1 and p == 3
                division(pic, p, otA, otB, last=last)
                if p == 3 and not last:
                    for it in range(pic * 4, pic * 4 + 4):
                        for ec in range(2):
                            fillers.append(mk_o(it, ec))

        for ic in range(NIC):
            if ic < NIC - 1:
                fillers.append(mk_q_dma(ic + 1))
                for io in range(4):
                    fillers.append(mk_q(ic + 1, io))
            for p in range(4):
                otA = otp.tile([P, 512], F32, tag="A", name="otA")
                otB = otp.tile([P, 512], F32, tag="B", name="otB")
                for jt in range(NJT):
                    ex = st_step(ic, p, jt)
                    pending.append((ex, otA, otB, p, jt, ic))
                    if ic == 0 and p == 0:
                        for _ in range(2):
                            if ic0_fill:
                                ic0_fill.popleft()()
                    elif step[0] % 2 == 0 and fillers:
                        fillers.popleft()()
                    step[0] += 1
                    # hold a new pair's first AVs a few extra steps so the
                    # single-buffered ot banks' drain (DVE copy of the
                    # previous pair) hides behind S^T work
                    for _ in range(2):
                        need = LAG + 3 if pending[0][4] == 0 else LAG
                        if len(pending) <= need:
                            break
                        flush_one()

        while pending:
            flush_one()
        while fillers:
            fillers.popleft()()
        # epilogue O-proj for the last i-chunk: pre-start four groups on
        # head-pairs 0-2 while the last division's broadcast bounce is in
        # flight; finish each with the pair-3 matmul once stg completes.
        og = [(it, ec) for it in range(12, 16) for ec in range(2)]
        pre = []
        for n, (it, ec) in enumerate(og[:4]):
            ps = o_psum(n)
            o_mm(ps, it, ec, 0, 3)
            pre.append((ps, it, ec))
        for n, (ps, it, ec) in enumerate(pre):
            o_mm(ps, it, ec, 3, 4)
            o_drain(ps, it, ec, act=n % 2 == 0)
        for n, (it, ec) in enumerate(og[4:]):
            ps = o_psum(n)
            o_mm(ps, it, ec, 0, 4)
            o_drain(ps, it, ec, act=n % 2 == 0)


def _legalize_sync(bir, max_waits=1, max_updates=1):
    """Split sync lists so every instruction carries at most `max_waits`
    waits and `max_updates` updates; the walrus build in this container
    rejects instructions with more ("Too many sync wait commands").
    Extra waits go on EventSemaphore instructions inserted just before
    (same engine => same program order), extra updates just after."""
    n = [0]

    def ev(engine, debug, waits, updates):
        n[0] += 1
        return {
            "debug": debug,
            "engine": engine,
            "ins": [],
            "outs": [],
            "name": f"I-syncsplit-{n[0]}",
            "opcode": "EventSemaphore",
            "sync_info": {"on_wait": waits, "on_update": updates},
        }

    for fn in bir["functions"]:
        for bb in fn["blocks"]:
            out = []
            for ins in bb["instructions"]:
                si = ins.get("sync_info")
                eng = ins.get("engine")
                post = []
                if si and eng:
                    waits = si.get("on_wait") or []
                    updates = si.get("on_update") or []
                    dbg = ins.get("debug", 0)
                    while len(waits) > max_waits:
                        chunk, waits = waits[:max_waits], waits[max_waits:]
                        out.append(ev(eng, dbg, chunk, []))
                    while len(updates) > max_updates:
                        updates, chunk = updates[:-max_updates], updates[-max_updates:]
                        post.append(ev(eng, dbg, [], chunk))
                    si["on_wait"] = waits
                    si["on_update"] = updates
                out.append(ins)
                out.extend(reversed(post))
            bb["instructions"] = out


_NC_CACHE = {}


def _get_nc():
    if "nc" not in _NC_CACHE:
        import json as _json

        nc = _build_kernel()
        orig = nc.to_json_bytes

        def patched():
            bir = _json.loads(orig())
            _legalize_sync(bir)
            return _json.dumps(bir).encode()

        nc.to_json_bytes = patched
        _NC_CACHE["nc"] = nc
    return _NC_CACHE["nc"]


def make_in_maps(q, k, v, attention_mask, Wq, bq, Wk, bk, Wv, bv, Wo, bo):
    """Host-side sharding: returns the per-core input maps."""
    import ml_dtypes

    bf = ml_dtypes.bfloat16
    q = np.asarray(q, np.float32)
    k = np.asarray(k, np.float32)
    v = np.asarray(v, np.float32)
    Wq = np.asarray(Wq, np.float32)
    Wk = np.asarray(Wk, np.float32)
    Wv = np.asarray(Wv, np.float32)
    Wo = np.asarray(Wo, np.float32)
    bq = np.asarray(bq, np.float32)
    bk = np.asarray(bk, np.float32)
    bv = np.asarray(bv, np.float32)
    mask = np.asarray(attention_mask)

    qTb = [np.ascontiguousarray(q[b].T).astype(bf) for b in range(4)]
    kTb = [np.ascontiguousarray(k[b].T).astype(bf) for b in range(4)]
    vTb = [np.ascontiguousarray(v[b].T).astype(bf) for b in range(4)]
    onec = np.ones((NHC,), np.float32)

    in_maps = []
    for c in range(8):
        bc, hg = c // 2, c % 2
        cs = slice(hg * C, (hg + 1) * C)
        in_maps.append({
            "qT": qTb[bc],
            "kT": kTb[bc],
            "vT": vTb[bc],
            "wq": np.ascontiguousarray(Wq[:, cs]).astype(bf),
            "wk": np.ascontiguousarray(Wk[:, cs]).astype(bf),
            "wv": np.ascontiguousarray(Wv[:, cs]).astype(bf),
            "wo": np.ascontiguousarray(Wo[cs, :]).astype(bf),
            "bq": np.ascontiguousarray(bq[cs]),
            "bk": np.ascontiguousarray(bk[cs]),
            "bv": np.ascontiguousarray(bv[cs]),
            "mm": mask[bc].astype(np.float32),
            "onec": onec,
        })
    return in_maps


def kernel(q, k, v, attention_mask, Wq, bq, Wk, bk, Wv, bv, Wo, bo, _trace=False):
    in_maps = make_in_maps(
        q, k, v, attention_mask, Wq, bq, Wk, bk, Wv, bv, Wo, bo
    )
    nc = _get_nc()
    import time as _time
    t0 = _time.time()
    try:
        res = run_bass_kernel_spmd(nc, in_maps, list(range(8)), trace=_trace)
    except Exception:
        if not _trace:
            raise
        res = run_bass_kernel_spmd(nc, in_maps, list(range(8)))
    kernel._last_run_seconds = _time.time() - t0
    bo = np.asarray(bo, np.float32)
    out = np.stack(
        [res.results[2 * b]["y"] + res.results[2 * b + 1]["y"] + bo
         for b in range(4)]
    ).astype(np.float32)
    if _trace:
        kernel._last_results = res
    return out


# revision 54
# speedup vs baseline: 1.2818x; 1.2818x over previous
"""Cross-attention kernel for Trainium2, 8 NeuronCores.

Problem: b=4, s=2048, d_model=1024, n_heads=16 (head_dim=64), fp32.
  out = softmax((q@Wq) (k@Wk)^T / sqrt(64) + mask) @ (v@Wv) @ Wo + bo

Sharding: core c handles batch c//2 and head-group c%2 (8 heads, 512
projection columns). Each core computes a partial output (s, 1024) =
(its heads' attention output) @ Wo[rows of its heads]; the host sums
the two partials per batch and adds bo.

v2 design (single fused pipeline; engine budget per core: PE ~1536
matmuls x 213ns = 327us serial-model / ~276us with row-tiling, ACT exp
33.5M elem = ~266us, fully overlapped; TimelineSim 361us vs 480us for
the phase-serial fp32r baseline):
  * All matmul operands bf16 (inputs converted host-side); PSUM fp32.
    bf16 operand rounding costs ~6e-3 rel err (gate 2e-2) and halves
    DMA + SBUF so everything stays resident.
  * S^T tiles for a HEAD PAIR computed concurrently via PE row tiling
    (contraction=64 each: head A rows 0-63, head B rows 64-127 of the
    array -> 2x PE throughput on hw; auto tile_position from the
    operands' base partitions), into one [128, 1024] PSUM pair tile ->
    one exp() per j-tile (256 ACT instructions of free-size 1024).
  * AV keeps the ones-column trick (M=65: numerator^T rows + softmax
    denominator row accumulated in the same matmul); V stored
    head-interleaved with mask-scaled ones columns written by DVE (no
    N=8 tail matmuls). AV trails S^T by LAG j-tiles (software
    pipeline; ex ring gives ACT runway across pair boundaries).
  * K/V/Q projections pipelined INTO the attention stream (j-group
    granularity for ic=0, first wk/xk quarter-split so PE starts ~4us
    in; Q(ic+1) and O-proj(ic) groups interleaved as PE filler via a
    deque popped every other step), AttnOut staged in SBUF, so both PE
    and ACT stay busy end to end.
  * Softmax division: ot PSUM copied to SBUF immediately (frees the
    single-buffered ot banks for the next pair), reciprocals of the
    denominator rows partition-broadcast via a DRAM bounce issued from
    gpsimd SWDGE (keeps sem-waits off the SP DMA FIFO), multiplies
    split DVE/gpsimd. qhT/khT/vh/stg are split per-chunk tiles so
    tile-granular dependency tracking doesn't serialize the pipeline.
  * Epilogue: the last pair's reciprocal broadcast runs as two K=1 PE
    matmuls into the idle st PSUM ring (no DRAM bounce on the critical
    path); last-chunk O-proj groups pre-start on head-pairs 0-2 across
    4 psum rings (pjp/st/otA/otB) under that division; drains
    alternate DVE and the by-then-idle ACT engine. y is written bf16
    (the host upcasts before summing the two per-batch partials).
PSUM: st pair tiles 2x[128,1024] (4 banks) + ot A/B 2x[128,512]
(2 banks, single-buffered) + proj/O-proj ring 2x[128,512] (2 banks).
"""

import collections

import numpy as np

import concourse.bass as bass
import concourse.tile as tile
from concourse import mybir
from concourse.bass_utils import run_bass_kernel_spmd

P = 128
S = 2048          # sequence length
DIN = 1024        # model dim
C = 512           # projection columns per core (8 heads * 64)
NHC = 8           # heads per core
HD = 64           # head dim
VW = NHC * (HD + 1)   # 520: head-interleaved V width incl. ones columns
NIC = S // 512    # 4 i-chunks
NJT = S // P      # 16 j-tiles
LAG = 9           # S^T -> AV software-pipeline distance (j-tiles)
F32 = mybir.dt.float32
BF16 = mybir.dt.bfloat16


def _build_kernel():
    nc = bass.Bass("TRN2", target_bir_lowering=False, debug=False)

    qT = nc.dram_tensor("qT", [DIN, S], BF16, kind="ExternalInput").ap()
    kT = nc.dram_tensor("kT", [DIN, S], BF16, kind="ExternalInput").ap()
    vT = nc.dram_tensor("vT", [DIN, S], BF16, kind="ExternalInput").ap()
    wq = nc.dram_tensor("wq", [DIN, C], BF16, kind="ExternalInput").ap()
    wk = nc.dram_tensor("wk", [DIN, C], BF16, kind="ExternalInput").ap()
    wv = nc.dram_tensor("wv", [DIN, C], BF16, kind="ExternalInput").ap()
    wo = nc.dram_tensor("wo", [C, DIN], BF16, kind="ExternalInput").ap()
    bq = nc.dram_tensor("bq", [C], F32, kind="ExternalInput").ap()
    bk = nc.dram_tensor("bk", [C], F32, kind="ExternalInput").ap()
    bv = nc.dram_tensor("bv", [C], F32, kind="ExternalInput").ap()
    mm = nc.dram_tensor("mm", [S], F32, kind="ExternalInput").ap()
    onec = nc.dram_tensor("onec", [NHC], F32, kind="ExternalInput").ap()
    y = nc.dram_tensor("y", [S, DIN], BF16, kind="ExternalOutput").ap()

    rcp_dram = nc.dram_tensor("rcp_st", [32, 512], F32).ap()  # 1/denom bounce

    with tile.TileContext(nc) as tc:
        _body(tc, y, rcp_dram, qT, kT, vT, wq, wk, wv, wo, bq, bk, bv, mm,
              onec)
    return nc


def _bcast_rows(ap, parts):
    """AP reading a 1-D (or row) DRAM region broadcast over `parts`
    partitions."""
    return bass.AP(tensor=ap.tensor, offset=ap.offset,
                   ap=[[0, parts]] + list(ap.ap))


def _blocks(ap2d, nblk, blk_stride, width, blk_off=0):
    """[p, nblk, width] AP over a 2-D [p, F] slice: blocks of `width`
    elements every `blk_stride`, starting at `blk_off`."""
    return bass.AP(tensor=ap2d.tensor, offset=ap2d.offset + blk_off,
                   ap=[list(ap2d.ap[0]), [blk_stride, nblk], [1, width]])


def _strided(ap2d, start, stride, count):
    """[p, count] AP: one element every `stride`, starting at `start`."""
    return bass.AP(tensor=ap2d.tensor, offset=ap2d.offset + start,
                   ap=[list(ap2d.ap[0]), [stride, count]])


def _xsrc(x_dram, col0, cols):
    """[128, 8, cols] AP over x_dram [1024, S]: partition = row%128,
    block = row//128, innermost = cols starting at col0."""
    return bass.AP(tensor=x_dram.tensor, offset=x_dram.offset + col0,
                   ap=[[S, P], [P * S, 8], [1, cols]])


def _body(tc, y, rcp_dram, qT, kT, vT, wq, wk, wv, wo, bq, bk, bv, mm, onec):
    nc = tc.nc

    with (
        tc.tile_pool(name="wpool", bufs=1) as wpool,
        tc.tile_pool(name="big", bufs=1) as big,
        tc.tile_pool(name="xin", bufs=2) as xin,
        tc.tile_pool(name="expool", bufs=8) as expool,
        tc.tile_pool(name="cppool", bufs=2) as cppool,
        tc.tile_pool(name="bcpool", bufs=2) as bcpool,
        tc.tile_pool(name="rcpp", bufs=2) as rcpp,
        tc.tile_pool(name="ytp", bufs=6) as ytp,
        tc.tile_pool(name="stp", bufs=2, space="PSUM") as stp,
        tc.tile_pool(name="otp", bufs=1, space="PSUM") as otp,
        tc.tile_pool(name="pjp", bufs=2, space="PSUM") as pjp,
    ):
        # ---------------- static tiles + weight DMAs ----------------
        # DMA transfers serialize in emission order (single-queue FIFO in
        # the model; shared HBM BW on hw), so emission order = arrival
        # priority: K-projection operands (wk+xk) gate the very first
        # matmuls, then Q, then V; wo can land tens of microseconds in.
        # wk/xk quarter-split (separate tiles): the very first K-proj
        # matmuls depend only on the first quarters, starting PE several
        # microseconds earlier under the serialized DMA model.
        wk_sb = []
        xk_parts = []
        for h in range(4):
            wkh = wpool.tile([P, 2, C], BF16, name=f"wk{h}")
            wk_sb.append(wkh)
            nc.sync.dma_start(
                out=wkh,
                in_=bass.AP(tensor=wk.tensor, offset=wk.offset + h * 2 * P * C,
                            ap=[[C, P], [P * C, 2], [1, C]]))
            xkh = xin.tile([P, 2, 512], BF16, tag=f"xk{h}", bufs=1,
                           name=f"xkp{h}")
            xk_parts.append(xkh)
            nc.sync.dma_start(
                out=xkh,
                in_=bass.AP(tensor=kT.tensor, offset=kT.offset + h * 2 * P * S,
                            ap=[[S, P], [P * S, 2], [1, 512]]))

        # split per i-chunk / j-group / head-pair: tile-granular RAW
        # tracking would otherwise serialize readers of one chunk behind
        # writers of another
        qhT = [big.tile([P, 4, 512], BF16, name=f"qhT{i}") for i in range(4)]
        khT = [big.tile([P, 4, 512], BF16, name=f"khT{i}") for i in range(4)]
        vh = [big.tile([P, 4, VW], BF16, name=f"vh{i}") for i in range(4)]
        stg = [big.tile([P, S], BF16, name=f"stg{i}") for i in range(4)]

        # ---------------- helper closures ----------------
        def dma_x(tag, x_dram, col0, cols=512):
            xt = xin.tile([P, 8, cols], BF16, tag=tag, name=f"x{tag}")
            nc.sync.dma_start(out=xt, in_=_xsrc(x_dram, col0, cols))
            return xt

        def qk_group(dst, wparts, b_sb, xparts, io):
            """dst[:, io, :] = (W[:, io-block]^T @ x) + bias.
            wparts/xparts: operand tiles, either (full,) with 8 kt-blocks
            or a (lo, hi) pair of 4-block halves."""
            ps = pjp.tile([P, 512], F32, name="pspj")
            nw, nx = 8 // len(wparts), 8 // len(xparts)
            for kt in range(8):
                w = wparts[kt // nw][:, kt % nw, io * P:(io + 1) * P]
                x = xparts[kt // nx][:, kt % nx, :]
                nc.tensor.matmul(
                    ps,
                    w,
                    x,
                    start=(kt == 0),
                    stop=(kt == 7),
                )
            nc.vector.tensor_scalar_add(
                out=dst[:, io, :],
                in0=ps,
                scalar1=b_sb[:, io:io + 1],
            )

        def v_group(jt, xt):
            """vh[:, jt, :] = interleave((x_jt @ Wv) + bv, ones) * mask"""
            ji = jt % 4
            ps = pjp.tile([P, 512], F32, name="pspj")
            for kt in range(8):
                nc.tensor.matmul(
                    ps,
                    (xt[:, kt, ji * P:(ji + 1) * P]),
                    (wv_sb[:, kt, :]),
                    start=(kt == 0),
                    stop=(kt == 7),
                )
            v2d = vh[jt // 4][:, jt % 4, :]
            numer = _blocks(v2d, NHC, HD + 1, HD)
            nc.vector.tensor_tensor(
                out=numer,
                in0=_blocks(ps, NHC, HD, HD),
                in1=_blocks(bvb, NHC, HD, HD),
                op=mybir.AluOpType.add,
            )
            nc.vector.tensor_scalar_mul(
                out=numer,
                in0=numer,
                scalar1=mm_sb[:, jt:jt + 1],
            )
            nc.vector.tensor_scalar_mul(
                out=_strided(v2d, HD, HD + 1, NHC),
                in0=ones8,
                scalar1=mm_sb[:, jt:jt + 1],
            )

        def st_step(ic, p, jt):
            """S^T pair tile for heads (2p, 2p+1) at (ic, jt) + exp."""
            st = stp.tile([P, 1024], F32, name="st")
            for u in range(2):
                nc.tensor.matmul(
                    st[:, u * 512:(u + 1) * 512],
                    (khT[jt // 4][u * HD:(u + 1) * HD, p,
                                  (jt % 4) * P:(jt % 4 + 1) * P]),
                    (qhT[ic][u * HD:(u + 1) * HD, p, :]),
                    start=True,
                    stop=True,
                )
            ex = expool.tile([P, 1024], BF16, name="ex")
            nc.scalar.activation(
                out=ex,
                in_=st,
                func=mybir.ActivationFunctionType.Exp,
                scale=float(HD) ** -0.5,
            )
            return ex

        def av_step(ex, otA, otB, p, jt):
            for u, ot in ((0, otA), (1, otB)):
                h = 2 * p + u
                nc.tensor.matmul(
                    ot[0:HD + 1, :],
                    (vh[jt // 4][:, jt % 4, h * (HD + 1):(h + 1) * (HD + 1)]),
                    (ex[:, u * 512:(u + 1) * 512]),
                    start=(jt == 0),
                    stop=(jt == NJT - 1),
                )

        def division(ic, p, otA, otB, last=False):
            """stg[:, p, ic-block] = numerators / denominator.

            Normally the full-tile copies run first so the single-buffered
            ot banks free ASAP (they gate the next pair's AV matmuls); on
            the last pair nothing follows, so the reciprocals read PSUM
            directly to start the DMA broadcast bounce sooner."""
            cA = cppool.tile([HD + 1, 512], F32, tag="cA", name="cA")
            cB = cppool.tile([HD + 1, 512], F32, tag="cB", name="cB")
            icb = slice(ic * 512, (ic + 1) * 512)
            if last:
                # epilogue fast path: broadcast the reciprocals over 64
                # partitions with two K=1 PE matmuls into the idle st PSUM
                # ring instead of the DRAM bounce (PE is the engine with
                # slack here; DMA handoffs are not).
                r2b = rcpp.tile([1, 1024], BF16, tag="r2b", bufs=1, name="r2b")
                with nc.allow_low_precision(reason="bf16 1/denom broadcast"):
                    nc.vector.reciprocal(out=r2b[:, 0:512],
                                         in_=otA[HD:HD + 1, :])
                    nc.vector.reciprocal(out=r2b[:, 512:1024],
                                         in_=otB[HD:HD + 1, :])
                nc.vector.tensor_copy(out=cA, in_=otA[0:HD + 1, :])
                nc.vector.tensor_copy(out=cB, in_=otB[0:HD + 1, :])
                bcp = stp.tile([HD, 1024], F32, tag="st", name="bcps")
                for u in range(2):
                    nc.tensor.matmul(
                        bcp[:, u * 512:(u + 1) * 512],
                        (ones64),
                        (r2b[:, u * 512:(u + 1) * 512]),
                        start=True,
                        stop=True,
                    )
                nc.vector.tensor_tensor(
                    out=stg[p][0:HD, icb], in0=cA[0:HD, :],
                    in1=bcp[:, 0:512], op=mybir.AluOpType.mult)
                nc.vector.tensor_tensor(
                    out=stg[p][HD:P, icb], in0=cB[0:HD, :],
                    in1=bcp[:, 512:1024], op=mybir.AluOpType.mult)
                return
            r2 = rcpp.tile([1, 1024], F32, name="r2")
            nc.vector.tensor_copy(out=cA, in_=otA[0:HD + 1, :])
            nc.vector.tensor_copy(out=cB, in_=otB[0:HD + 1, :])
            nc.vector.reciprocal(out=r2[:, 0:512], in_=cA[HD:HD + 1, :])
            nc.vector.reciprocal(out=r2[:, 512:1024], in_=cB[HD:HD + 1, :])
            slot = 2 * (ic * 4 + p)
            rows = rcp_dram[slot:slot + 2, :]
            # SWDGE (gpsimd-issued) DMAs mid-stream: the sem-wait on the
            # reciprocals would otherwise block the SP DMA-issue FIFO,
            # delaying every bulk transfer queued behind it.
            nc.gpsimd.dma_start(out=rows, in_=r2)
            bc = bcpool.tile([HD, 1024], F32, name="bc")
            # one DMA: each denominator row broadcast over 64 partitions
            # (A's in cols 0-511, B's in 512-1023, both at base partition 0)
            nc.gpsimd.dma_start(
                out=bc,
                in_=bass.AP(tensor=rows.tensor, offset=rows.offset,
                            ap=[[0, HD], [512, 2], [1, 512]]))
            # one multiply on DVE, one on the otherwise-idle Pool engine:
            # they run in parallel, halving the division tail.
            nc.vector.tensor_tensor(
                out=stg[p][0:HD, icb], in0=cA[0:HD, :], in1=bc[:, 0:512],
                op=mybir.AluOpType.mult)
            nc.gpsimd.tensor_tensor(
                out=stg[p][HD:P, icb], in0=cB[0:HD, :], in1=bc[:, 512:1024],
                op=mybir.AluOpType.mult)

        def o_mm(ps, it, ec, ft0, ft1):
            for ft in range(ft0, ft1):
                nc.tensor.matmul(
                    ps,
                    (stg[ft][:, it * P:(it + 1) * P]),
                    (wo_sb[:, ft, ec * 512:(ec + 1) * 512]),
                    start=(ft == 0),
                    stop=(ft == 3),
                )

        def o_drain(ps, it, ec, act=False):
            """act=True drains via the scalar engine (idle in the epilogue;
            Copy is in every activation table set so no table reload)."""
            yt = ytp.tile([P, 512], BF16, name="yt")
            if act:
                nc.scalar.activation(
                    out=yt, in_=ps, func=mybir.ActivationFunctionType.Copy)
                # issue from the ACT queue: zero sem-wait right after the
                # Copy, and a second parallel DMA-issue path in the tail
                nc.scalar.dma_start(
                    out=y[it * P:(it + 1) * P, ec * 512:(ec + 1) * 512],
                    in_=yt)
            else:
                nc.vector.tensor_copy(out=yt, in_=ps)
                nc.sync.dma_start(
                    out=y[it * P:(it + 1) * P, ec * 512:(ec + 1) * 512],
                    in_=yt)

        def o_psum(n):
            if n % 2 == 0:
                # reuse the st tag's (idle in the epilogue) ring slots
                return stp.tile([P, 512], F32, name="pso", tag="st")
            return pjp.tile([P, 512], F32, name="pspj")

        def o_psum_ep(n):
            # epilogue: also borrow the freed ot banks -> 4 independent slots
            kind = ("st", "pj", "A", "B")[n % 4]
            if kind == "st":
                return stp.tile([P, 512], F32, name="pso", tag="st")
            if kind == "pj":
                return pjp.tile([P, 512], F32, name="pspj")
            return otp.tile([P, 512], F32, name=f"pso{kind}", tag=kind)

        def o_group(it, ec, act=False):
            """y[it-block, ec-block] = AttnOut[it] @ Wo[:, ec-block]."""
            ps = o_psum(1 if not act else 0)
            o_mm(ps, it, ec, 0, 4)
            o_drain(ps, it, ec, act)

        # ---------------- prologue ----------------
        bk_sb = wpool.tile([P, 4], F32)
        nc.sync.dma_start(out=bk_sb, in_=bk.rearrange("(t p) -> p t", p=P))
        wq_sb = wpool.tile([P, 8, C], BF16)
        nc.sync.dma_start(out=wq_sb, in_=wq.rearrange("(t p) c -> p t c", p=P))
        xq = dma_x("xq", qT, 0)
        bq_sb = wpool.tile([P, 4], F32)
        nc.sync.dma_start(out=bq_sb, in_=bq.rearrange("(t p) -> p t", p=P))
        wv_sb = wpool.tile([P, 8, C], BF16)
        nc.sync.dma_start(out=wv_sb, in_=wv.rearrange("(t p) c -> p t c", p=P))
        xv = dma_x("xv", vT, 0)
        bvb = wpool.tile([P, C], F32)
        nc.sync.dma_start(out=bvb, in_=_bcast_rows(bv, P))
        mm_sb = wpool.tile([P, NJT], F32)
        nc.sync.dma_start(out=mm_sb, in_=mm.rearrange("(t p) -> p t", p=P))
        ones8 = wpool.tile([P, NHC], F32)
        nc.sync.dma_start(out=ones8, in_=_bcast_rows(onec, P))
        ones64 = wpool.tile([1, HD], BF16)
        nc.vector.memset(ones64, 1.0)
        wo_sb = wpool.tile([P, 4, DIN], BF16)
        nc.sync.dma_start(out=wo_sb, in_=wo.rearrange("(t p) c -> p t c", p=P))
        for io in range(4):
            qk_group(khT[0], wk_sb, bk_sb, xk_parts, io)
        for io in range(4):
            qk_group(qhT[0], (wq_sb,), bq_sb, (xq,), io)
        for jt in range(4):
            v_group(jt, xv)

        # ---------------- fused attention pipeline ----------------
        fillers = collections.deque()
        # ic0/pair0 K+V projection stream, consumed 2 per step (8 groups
        # per j-group of 4 steps -> each j-group ready exactly in time).
        kvx = [None, None, None, None]

        def mk_kv_dma(jg):
            def f():
                kvx[jg] = (dma_x("xk", kT, jg * 512), dma_x("xv", vT, jg * 512))
            return f

        def mk_k(jg, io):
            return lambda: qk_group(khT[jg], wk_sb, bk_sb, (kvx[jg][0],), io)

        def mk_v(jg, jt):
            return lambda: v_group(jt, kvx[jg][1])

        ic0_fill = collections.deque()
        for jg in range(1, 4):
            ic0_fill.append(mk_kv_dma(jg))
            for io in range(4):
                ic0_fill.append(mk_k(jg, io))
            for jt in range(jg * 4, jg * 4 + 4):
                ic0_fill.append(mk_v(jg, jt))

        qx = [None]

        def mk_q_dma(ic):
            def f():
                qx[0] = dma_x("xq", qT, ic * 512)
            return f

        def mk_q(ic, io):
            return lambda: qk_group(qhT[ic], (wq_sb,), bq_sb, (qx[0],), io)

        def mk_o(it, ec, act=False):
            return lambda: o_group(it, ec, act)

        pending = collections.deque()  # (ex, otA, otB, p, jt, ic)
        step = [0]

        def flush_one():
            ex, otA, otB, p, jt, pic = pending.popleft()
            av_step(ex, otA, otB, p, jt)
            if jt == NJT - 1:
                last = pic == NIC - 1 and p == 3
                division(pic, p, otA, otB, last=last)
                if p == 3 and not last:
                    for it in range(pic * 4, pic * 4 + 4):
                        for ec in range(2):
                            fillers.append(mk_o(it, ec))

        for ic in range(NIC):
            if ic < NIC - 1:
                fillers.append(mk_q_dma(ic + 1))
                for io in range(4):
                    fillers.append(mk_q(ic + 1, io))
            for p in range(4):
                otA = otp.tile([P, 512], F32, tag="A", name="otA")
                otB = otp.tile([P, 512], F32, tag="B", name="otB")
                for jt in range(NJT):
                    ex = st_step(ic, p, jt)
                    pending.append((ex, otA, otB, p, jt, ic))
                    if ic == 0 and p == 0:
                        for _ in range(2):
                            if ic0_fill:
                                ic0_fill.popleft()()
                    elif step[0] % 2 == 0 and fillers:
                        fillers.popleft()()
                    step[0] += 1
                    # hold a new pair's first AVs a few extra steps so the
                    # single-buffered ot banks' drain (DVE copy of the
                    # previous pair) hides behind S^T work
                    for _ in range(2):
                        need = LAG + 0 if pending[0][4] == 0 else LAG
                        if len(pending) <= need:
                            break
                        flush_one()

        while pending:
            flush_one()
        while fillers:
            fillers.popleft()()
        # epilogue O-proj for the last i-chunk: pre-start four groups on
        # head-pairs 0-2 while the last division's broadcast bounce is in
        # flight; finish each with the pair-3 matmul once stg completes.
        og = [(it, ec) for it in range(12, 16) for ec in range(2)]
        pre = []
        for n, (it, ec) in enumerate(og[:4]):
            ps = o_psum_ep(n)
            o_mm(ps, it, ec, 0, 3)
            pre.append((ps, it, ec))
        for n, (ps, it, ec) in enumerate(pre):
            o_mm(ps, it, ec, 3, 4)
            o_drain(ps, it, ec, act=n % 2 == 0)
        for n, (it, ec) in enumerate(og[4:]):
            ps = o_psum_ep(n)
            o_mm(ps, it, ec, 0, 4)
            o_drain(ps, it, ec, act=n % 2 == 0)


def _legalize_sync(bir, max_waits=1, max_updates=1):
    """Split sync lists so every instruction carries at most `max_waits`
    waits and `max_updates` updates; the walrus build in this container
    rejects instructions with more ("Too many sync wait commands").
    Extra waits go on EventSemaphore instructions inserted just before
    (same engine => same program order), extra updates just after."""
    n = [0]

    def ev(engine, debug, waits, updates):
        n[0] += 1
        return {
            "debug": debug,
            "engine": engine,
            "ins": [],
            "outs": [],
            "name": f"I-syncsplit-{n[0]}",
            "opcode": "EventSemaphore",
            "sync_info": {"on_wait": waits, "on_update": updates},
        }

    for fn in bir["functions"]:
        for bb in fn["blocks"]:
            out = []
            for ins in bb["instructions"]:
                si = ins.get("sync_info")
                eng = ins.get("engine")
                post = []
                if si and eng:
                    waits = si.get("on_wait") or []
                    updates = si.get("on_update") or []
                    dbg = ins.get("debug", 0)
                    while len(waits) > max_waits:
                        chunk, waits = waits[:max_waits], waits[max_waits:]
                        out.append(ev(eng, dbg, chunk, []))
                    while len(updates) > max_updates:
                        updates, chunk = updates[:-max_updates], updates[-max_updates:]
                        post.append(ev(eng, dbg, [], chunk))
                    si["on_wait"] = waits
                    si["on_update"] = updates
                out.append(ins)
                out.extend(reversed(post))
            bb["instructions"] = out


_NC_CACHE = {}


def _get_nc():
    if "nc" not in _NC_CACHE:
        import json as _json

        nc = _build_kernel()
        orig = nc.to_json_bytes

        def patched():
            bir = _json.loads(orig())
            _legalize_sync(bir)
            return _json.dumps(bir).encode()

        nc.to_json_bytes = patched
        _NC_CACHE["nc"] = nc
    return _NC_CACHE["nc"]


def make_in_maps(q, k, v, attention_mask, Wq, bq, Wk, bk, Wv, bv, Wo, bo):
    """Host-side sharding: returns the per-core input maps."""
    import ml_dtypes

    bf = ml_dtypes.bfloat16
    q = np.asarray(q, np.float32)
    k = np.asarray(k, np.float32)
    v = np.asarray(v, np.float32)
    Wq = np.asarray(Wq, np.float32)
    Wk = np.asarray(Wk, np.float32)
    Wv = np.asarray(Wv, np.float32)
    Wo = np.asarray(Wo, np.float32)
    bq = np.asarray(bq, np.float32)
    bk = np.asarray(bk, np.float32)
    bv = np.asarray(bv, np.float32)
    mask = np.asarray(attention_mask)

    # single-pass transpose+downcast (astype with order='C' fuses both)
    qTb = [q[b].T.astype(bf, order="C") for b in range(4)]
    kTb = [k[b].T.astype(bf, order="C") for b in range(4)]
    vTb = [v[b].T.astype(bf, order="C") for b in range(4)]
    onec = np.ones((NHC,), np.float32)

    in_maps = []
    for c in range(8):
        bc, hg = c // 2, c % 2
        cs = slice(hg * C, (hg + 1) * C)
        in_maps.append({
            "qT": qTb[bc],
            "kT": kTb[bc],
            "vT": vTb[bc],
            "wq": Wq[:, cs].astype(bf, order="C"),
            "wk": Wk[:, cs].astype(bf, order="C"),
            "wv": Wv[:, cs].astype(bf, order="C"),
            "wo": Wo[cs, :].astype(bf, order="C"),
            "bq": np.ascontiguousarray(bq[cs]),
            "bk": np.ascontiguousarray(bk[cs]),
            "bv": np.ascontiguousarray(bv[cs]),
            "mm": mask[bc].astype(np.float32),
            "onec": onec,
        })
    return in_maps


def kernel(q, k, v, attention_mask, Wq, bq, Wk, bk, Wv, bv, Wo, bo, _trace=False):
    in_maps = make_in_maps(
        q, k, v, attention_mask, Wq, bq, Wk, bk, Wv, bv, Wo, bo
    )
    nc = _get_nc()
    import time as _time
    t0 = _time.time()
    try:
        res = run_bass_kernel_spmd(nc, in_maps, list(range(8)), trace=_trace)
    except Exception:
        if not _trace:
            raise
        res = run_bass_kernel_spmd(nc, in_maps, list(range(8)))
    kernel._last_run_seconds = _time.time() - t0
    bo = np.asarray(bo, np.float32)
    out = np.stack(
        [res.results[2 * b]["y"].astype(np.float32)
         + res.results[2 * b + 1]["y"].astype(np.float32) + bo
         for b in range(4)]
    ).astype(np.float32)
    if _trace:
        kernel._last_results = res
    return out
